# revision 1
# baseline (speedup 1.0000x reference)
"""Trainium2 Bass kernel for nn_DocREModel (DocRE-style relation extraction head).

Strategy (8 NeuronCores, two SPMD launches):

Launch 1  (core c -> batch b=c//4, l-slice q=c%4 of 256 positions):
  - dma_gather the mention rows of `attention[b,:,:,lslice]` (the ragged gather),
    masked-mean over mentions via a block-diagonal matmul -> ent_att E.
  - transpose E to l-major, compute upper-triangular pair products
    G[u,l] = sum_h E[i,h,l]*E[j,h,l] on the vector engine.
  - seqW = seq[b,lslice] @ [W_lin | 1]/H  (PE), then partial
    ai[u,:] = G @ seqW (PE).  ai[:, :3] = unnormalized feature.W_lin, ai[:,3] = rowsum.
  - mention-gather of sequence_output rows + masked logsumexp -> ent_emb^T.
  Outputs: ai_part [1024,4], ent_T [768,48].  Host sums ai partials per batch
  (pure resharding glue) and expands the unique-pair table to hts order.

Launch 2  (core c -> channel slice of 13 of the 97 bilinear output channels):
  - normalize ai by rowsum (the ht_att normalization), h_t = relu(ai' @ W_seg_aug),
  - P_head/P_tail = [ent_emb;1] @ W_{head,tail}_aug (bias folded),
  - hs = tanh(h_t + onehot_h @ P_head) (pair-major),
    ts^T = tanh(h_t^T + P_tail^T-gather) produced directly transposed,
  - bilinear: per pair-tile, R = ts^T.T @ W_bil^T-slice on PE (contraction over j),
    then logits[p,o] = sum_i hs[p,i]*R[p,(o,i)] via fused DVE tensor_tensor_reduce
    reading R straight from PSUM.
  Output: logits_part [3456,13]; host concatenates channel slices.
"""

import os
import sys

for _p in ("/opt/trn_rl_repo", "/root/.axon_site/_ro/trn_rl_repo"):
    if os.path.isdir(_p) and _p not in sys.path:
        sys.path.append(_p)

import numpy as np
from ml_dtypes import bfloat16 as np_bf16

from concourse import bacc, bass, mybir, tile
from concourse import bass_utils

F32 = mybir.dt.float32
F32R = mybir.dt.float32r
BF16 = mybir.dt.bfloat16
I16 = mybir.dt.int16
ALU = mybir.AluOpType
ACTF = mybir.ActivationFunctionType

# Problem shape (hardcoded per the harness contract).
B, L, D, H, NE, MM, NP, C, F2 = 2, 1024, 768, 12, 42, 8, 1722, 97, 256
NCORES = 8
LS = L // 4            # 256: l-slice per launch-1 core
NEP = 48               # padded entity count (3 groups of 16)
NG = NE // 16 + 1      # 3 ne-groups
NU = NE * (NE + 1) // 2  # 903 unique unordered pairs
NU_PAD = 1024
P3 = B * NP            # 3444 pairs total
P3_PAD = 3456          # 27 tiles of 128
PT = P3_PAD // 128     # 27
NO = 13                # channels per core (8*13 = 104 >= 97)
KD = D // 128          # 6 k-tiles over D
DA = 896               # augmented D (768 + bias row, padded to 7*128)
BN = 2 * NEP           # 96 (batch, entity) rows

# Upper-tri pair ordering: u(d, i) = OFF_D[d] + i, pair = (i, i+d), d in [0,42)
OFF_D = np.concatenate([[0], np.cumsum(NE - np.arange(NE))]).astype(np.int64)


def _pair_u(a, b_):
    i = np.minimum(a, b_)
    d = np.abs(a - b_)
    return OFF_D[d] + i


def _wrap_idx16(idx, n):
    """Pack indices into the [128, n//16] int16 layout dma_gather expects
    (index d lives at [d % 16, d // 16]; rows replicated to 128 partitions)."""
    assert len(idx) == n and n % 16 == 0
    out = np.zeros((16, n // 16), dtype=np.int16)
    out[np.arange(n) % 16, np.arange(n) // 16] = idx
    return np.tile(out, (8, 1))


# ---------------------------------------------------------------------------
# Launch 1 program
# ---------------------------------------------------------------------------

def build_launch1():
    nc = bacc.Bacc("TRN2", target_bir_lowering=False, debug=False)
    att = nc.declare_dram_parameter("att", [L, H * LS], BF16, isOutput=False)
    seq = nc.declare_dram_parameter("seq", [L, D], F32, isOutput=False)
    seqT = nc.declare_dram_parameter("seqT", [D, LS], F32, isOutput=False)
    wlin = nc.declare_dram_parameter("wlin", [D, 4], F32, isOutput=False)
    wmsk = nc.declare_dram_parameter("wmsk", [128, NEP], BF16, isOutput=False)
    amask = nc.declare_dram_parameter("amask", [128, NEP * MM], F32, isOutput=False)
    midx = nc.declare_dram_parameter("midx", [128, NG * 128 // 16], I16, isOutput=False)
    ident = nc.declare_dram_parameter("ident", [128, 128], F32, isOutput=False)
    identb = nc.declare_dram_parameter("identb", [128, 128], BF16, isOutput=False)
    ai_out = nc.declare_dram_parameter("ai_part", [NU_PAD, 4], F32, isOutput=True)
    ent_out = nc.declare_dram_parameter("ent_T", [D, NEP], F32, isOutput=True)

    NMEN = NG * 128  # 384 gathered rows (attention and sequence share idxs)

    with tile.TileContext(nc) as tc:
        with (
            tc.tile_pool(name="big", bufs=1) as big,
            tc.tile_pool(name="small", bufs=1) as small,
            tc.tile_pool(name="work", bufs=2) as work,
            tc.tile_pool(name="psum", bufs=2, space="PSUM") as psum,
        ):
            # ---- input loads ----
            att_rows = big.tile([128, NG * H * LS], BF16)
            ment_rows = big.tile([128, NG * D], F32)
            seqT_sb = big.tile([128, KD * LS], F32)
            wlin_sb = small.tile([128, KD * 4], F32)
            wmsk_sb = small.tile([128, NEP], BF16)
            amask_sb = small.tile([128, NEP * MM], F32)
            midx_sb = small.tile([128, NMEN // 16], I16)
            ident_sb = small.tile([128, 128], F32)
            identb_sb = small.tile([128, 128], BF16)

            nc.sync.dma_start(out=seqT_sb[:].rearrange("p (k l) -> p k l", k=KD),
                              in_=seqT[:].rearrange("(k p) l -> p k l", p=128))
            nc.sync.dma_start(out=wlin_sb[:].rearrange("p (k x) -> p k x", k=KD),
                              in_=wlin[:].rearrange("(k p) x -> p k x", p=128))
            nc.sync.dma_start(out=wmsk_sb[:], in_=wmsk[:])
            nc.sync.dma_start(out=amask_sb[:], in_=amask[:])
            nc.sync.dma_start(out=midx_sb[:], in_=midx[:])
            nc.sync.dma_start(out=ident_sb[:], in_=ident[:])
            nc.sync.dma_start(out=identb_sb[:], in_=identb[:])

            # ---- the two gathers (descriptor-cheap SWDGE) ----
            nc.gpsimd.dma_gather(
                out_ap=att_rows[:].rearrange("p (c l) -> p c l", l=H * LS),
                in_ap=att[:], idxs_ap=midx_sb[:],
                num_idxs=NMEN, num_idxs_reg=NMEN, elem_size=H * LS,
                single_packet=False)
            nc.gpsimd.dma_gather(
                out_ap=ment_rows[:].rearrange("p (c l) -> p c l", l=D),
                in_ap=seq[:], idxs_ap=midx_sb[:],
                num_idxs=NMEN, num_idxs_reg=NMEN, elem_size=D,
                single_packet=False)

            # ---- masked mean over mentions: E_g[ne_sub, (h,l)] per group ----
            E_g = [big.tile([16, H * LS], BF16, name=f"E_g{g}") for g in range(NG)]
            for g in range(NG):
                for ch in range(6):  # pairs of heads -> N=512
                    eps = psum.tile([16, 2 * LS], F32, space="PSUM", tag="ps")
                    rhs = att_rows[:, g * H * LS + 2 * ch * LS:
                                   g * H * LS + (2 * ch + 2) * LS]
                    nc.tensor.matmul(eps[:],
                                     lhsT=wmsk_sb[:, g * 16:(g + 1) * 16],
                                     rhs=rhs, start=True, stop=True)
                    nc.any.tensor_copy(
                        E_g[g][:, 2 * ch * LS:(2 * ch + 2) * LS], eps[:])

            # ---- transpose E -> E_T[lt][l, (h, ne)] ----
            E_T = [big.tile([128, H * NEP], BF16, name=f"E_T{lt}") for lt in range(2)]
            for h in range(H):
                for lt in range(2):
                    for g in range(NG):
                        tps = psum.tile([128, 16], BF16, space="PSUM", tag="psb")
                        nc.tensor.transpose(
                            tps[:],
                            E_g[g][:, h * LS + lt * 128: h * LS + (lt + 1) * 128],
                            identb_sb[:16, :16])
                        nc.any.tensor_copy(
                            E_T[lt][:, h * NEP + g * 16: h * NEP + (g + 1) * 16],
                            tps[:])

            # ---- upper-tri pair products G_T[l, u] ----
            G_T = [big.tile([128, NU_PAD], F32, name=f"G_T{lt}") for lt in range(2)]
            for lt in range(2):
                nc.vector.memset(G_T[lt][:, NU:], 0.0)
                ev = E_T[lt][:].rearrange("p (h i) -> p h i", h=H)
                for d in range(NE):
                    n = NE - d
                    tmpG = work.tile([128, 504], BF16, tag="tmpG")
                    in0 = ev[:, :, 0:n].transpose([0, 2, 1])
                    in1 = ev[:, :, d:d + n].transpose([0, 2, 1])
                    prod = tmpG[:, :n * H].rearrange("p (i h) -> p i h", h=H)
                    nc.vector.tensor_tensor(out=prod, in0=in0, in1=in1, op=ALU.mult)
                    nc.vector.tensor_reduce(
                        out=G_T[lt][:, OFF_D[d]:OFF_D[d] + n], in_=prod,
                        axis=mybir.AxisListType.X, op=ALU.add)

            # ---- seqW = seqT.T @ [W_lin|e]/H ----
            seqW = [small.tile([128, 4], F32, name=f"seqW{lt}") for lt in range(2)]
            for lt in range(2):
                swps = psum.tile([128, 4], F32, space="PSUM", tag="ps")
                for kt in range(KD):
                    nc.tensor.matmul(
                        swps[:],
                        lhsT=seqT_sb[:, kt * LS + lt * 128: kt * LS + (lt + 1) * 128],
                        rhs=wlin_sb[:, kt * 4:(kt + 1) * 4],
                        start=(kt == 0), stop=(kt == KD - 1))
                nc.scalar.activation(seqW[lt][:], swps[:], ACTF.Copy, scale=1.0 / H)
                nc.vector.memset(seqW[lt][:, 3:4], 1.0 / H)

            # ---- partial ai = G_T.T @ seqW ----
            ai_sb = small.tile([128, 8 * 4], F32)
            for uc in range(8):
                aps = psum.tile([128, 4], F32, space="PSUM", tag="ps")
                for lt in range(2):
                    nc.tensor.matmul(
                        aps[:], lhsT=G_T[lt][:, uc * 128:(uc + 1) * 128],
                        rhs=seqW[lt][:], start=(lt == 0), stop=(lt == 1))
                nc.any.tensor_copy(ai_sb[:, uc * 4:(uc + 1) * 4], aps[:])
            nc.sync.dma_start(
                out=ai_out[:].rearrange("(c p) x -> p c x", p=128),
                in_=ai_sb[:].rearrange("p (c x) -> p c x", x=4))

            # ---- mention transposes + masked logsumexp -> ent_T ----
            ent_sb = big.tile([128, KD * NEP], F32)
            for dt in range(KD):
                mT = work.tile([128, NG * 128], F32, tag="mT")
                for g in range(NG):
                    mps = psum.tile([128, 128], F32, space="PSUM", tag="ps")
                    nc.tensor.transpose(
                        mps[:], ment_rows[:, g * D + dt * 128: g * D + (dt + 1) * 128],
                        ident_sb[:])
                    nc.any.tensor_copy(mT[:, g * 128:(g + 1) * 128], mps[:])
                # masked logsumexp over m (innermost, 8 slots)
                xm = work.tile([128, NEP * MM], F32, tag="xm")
                nc.vector.tensor_tensor(out=xm[:], in0=mT[:],
                                        in1=amask_sb[:],
                                        op=ALU.add)
                xmv = xm[:].rearrange("p (e m) -> p e m", m=MM)
                mx = work.tile([128, NEP], F32, tag="mx")
                nc.vector.tensor_reduce(out=mx[:], in_=xmv,
                                        axis=mybir.AxisListType.X, op=ALU.max)
                xs = work.tile([128, NEP * MM], F32, tag="xs")
                nc.vector.tensor_tensor(
                    out=xs[:].rearrange("p (e m) -> p e m", m=MM), in0=xmv,
                    in1=mx[:].unsqueeze(2).to_broadcast([128, NEP, MM]),
                    op=ALU.subtract)
                es = work.tile([128, NEP * MM], F32, tag="es")
                nc.scalar.activation(es[:], xs[:], ACTF.Exp)
                sm = work.tile([128, NEP], F32, tag="sm")
                nc.vector.tensor_reduce(
                    out=sm[:], in_=es[:].rearrange("p (e m) -> p e m", m=MM),
                    axis=mybir.AxisListType.X, op=ALU.add)
                ln = work.tile([128, NEP], F32, tag="ln")
                nc.scalar.activation(ln[:], sm[:], ACTF.Ln)
                nc.vector.tensor_tensor(
                    out=ent_sb[:, dt * NEP:(dt + 1) * NEP], in0=ln[:], in1=mx[:],
                    op=ALU.add)
            nc.sync.dma_start(
                out=ent_out[:].rearrange("(k p) e -> p k e", p=128),
                in_=ent_sb[:].rearrange("p (k e) -> p k e", e=NEP))
    nc.compile()
    return nc


# ---------------------------------------------------------------------------
# Launch 2 program
# ---------------------------------------------------------------------------

def build_launch2():
    nc = bacc.Bacc("TRN2", target_bir_lowering=False, debug=False)
    aip = nc.declare_dram_parameter("ai_pairs", [P3_PAD, 4], F32, isOutput=False)
    entA = nc.declare_dram_parameter("entA", [DA, BN], F32, isOutput=False)
    whead = nc.declare_dram_parameter("whead", [DA, F2], F32, isOutput=False)
    wtail = nc.declare_dram_parameter("wtail", [DA, F2], F32, isOutput=False)
    wseg = nc.declare_dram_parameter("wseg", [4, F2], F32, isOutput=False)
    oh_h = nc.declare_dram_parameter("oh_h", [BN, P3_PAD], F32, isOutput=False)
    oh_t = nc.declare_dram_parameter("oh_t", [BN, P3_PAD], F32, isOutput=False)
    wbil = nc.declare_dram_parameter("wbil", [F2, NO * F2], BF16, isOutput=False)
    bbil = nc.declare_dram_parameter("bbil", [128, NO], F32, isOutput=False)
    ident = nc.declare_dram_parameter("ident", [128, 128], F32, isOutput=False)
    lg_out = nc.declare_dram_parameter("logits_part", [P3_PAD, NO], F32,
                                       isOutput=True)
    KA = DA // 128  # 7

    with tile.TileContext(nc) as tc:
        with (
            tc.tile_pool(name="big", bufs=1) as big,
            tc.tile_pool(name="small", bufs=1) as small,
            tc.tile_pool(name="work", bufs=2) as work,
            tc.tile_pool(name="psum", bufs=2, space="PSUM") as psum,
            tc.tile_pool(name="rpsum", bufs=3, space="PSUM") as rpsum,
        ):
            ai_sb = small.tile([128, PT * 4], F32)
            entA_sb = big.tile([128, KA * BN], F32)
            wh_sb = big.tile([128, KA * F2], F32)
            wt_sb = big.tile([128, KA * F2], F32)
            wseg_sb = small.tile([4, F2], F32)
            ohh_sb = big.tile([BN, P3_PAD], F32)
            oht_sb = big.tile([BN, P3_PAD], F32)
            wbil_sb = [big.tile([128, NO * F2], BF16, name=f"wbil{j}")
                       for j in range(2)]
            bbil_sb = small.tile([128, NO], F32)
            ident_sb = small.tile([128, 128], F32)

            nc.sync.dma_start(out=ai_sb[:].rearrange("p (t x) -> p t x", x=4),
                              in_=aip[:].rearrange("(t p) x -> p t x", p=128))
            nc.sync.dma_start(out=entA_sb[:].rearrange("p (k n) -> p k n", k=KA),
                              in_=entA[:].rearrange("(k p) n -> p k n", p=128))
            nc.sync.dma_start(out=wh_sb[:].rearrange("p (k f) -> p k f", k=KA),
                              in_=whead[:].rearrange("(k p) f -> p k f", p=128))
            nc.sync.dma_start(out=wt_sb[:].rearrange("p (k f) -> p k f", k=KA),
                              in_=wtail[:].rearrange("(k p) f -> p k f", p=128))
            nc.sync.dma_start(out=wseg_sb[:], in_=wseg[:])
            nc.sync.dma_start(out=ohh_sb[:], in_=oh_h[:])
            nc.sync.dma_start(out=oht_sb[:], in_=oh_t[:])
            for j in range(2):
                nc.sync.dma_start(
                    out=wbil_sb[j][:],
                    in_=wbil[j * 128:(j + 1) * 128, :])
            nc.sync.dma_start(out=bbil_sb[:], in_=bbil[:])
            nc.sync.dma_start(out=ident_sb[:], in_=ident[:])

            # ---- normalize ai by rowsum (ht_att normalization) ----
            aiv = ai_sb[:].rearrange("p (t x) -> p t x", x=4)
            rsum = small.tile([128, PT], F32)
            nc.vector.tensor_scalar_add(rsum[:], aiv[:, :, 3], 1e-5)
            rinv = small.tile([128, PT], F32)
            nc.vector.reciprocal(rinv[:], rsum[:])
            for x in range(3):
                nc.vector.tensor_tensor(out=aiv[:, :, x], in0=aiv[:, :, x],
                                        in1=rinv[:], op=ALU.mult)
            nc.vector.memset(aiv[:, :, 3], 1.0)

            # ---- transpose ai tiles -> aiT [4, P3_PAD] ----
            aiT = small.tile([4, P3_PAD], F32)
            for t in range(PT):
                tps = psum.tile([4, 128], F32, space="PSUM", tag="ps")
                nc.tensor.transpose(tps[:], ai_sb[:, t * 4:(t + 1) * 4],
                                    ident_sb[:])
                nc.any.tensor_copy(aiT[:, t * 128:(t + 1) * 128], tps[:])

            # ---- h_t pair-major [p, F2] ----
            h_t = big.tile([128, PT * F2], F32)
            for t in range(PT):
                hps = psum.tile([128, F2], F32, space="PSUM", tag="ps")
                nc.tensor.matmul(hps[:],
                                 lhsT=aiT[:, t * 128:(t + 1) * 128],
                                 rhs=wseg_sb[:],
                                 start=True, stop=True)
                nc.scalar.activation(h_t[:, t * F2:(t + 1) * F2], hps[:], ACTF.Relu)

            # ---- h_t transposed [f, p] ----
            h_tT = [big.tile([128, P3_PAD], F32, name=f"h_tT{m}") for m in range(2)]
            for m in range(2):
                for nchk in range(PT // 4 + 1):  # 7 chunks of <=512
                    n0, n1 = nchk * 512, min((nchk + 1) * 512, P3_PAD)
                    if n0 >= n1:
                        continue
                    hps2 = psum.tile([128, 512], F32, space="PSUM", tag="ps")
                    nc.tensor.matmul(hps2[:, :n1 - n0],
                                     lhsT=wseg_sb[:, m * 128:(m + 1) * 128],
                                     rhs=aiT[:, n0:n1],
                                     start=True, stop=True)
                    nc.scalar.activation(h_tT[m][:, n0:n1], hps2[:, :n1 - n0],
                                         ACTF.Relu)

            # ---- projections P_head/P_tail [bn, F2] ----
            proj = {}
            for nm, w_sb in (("h", wh_sb), ("t", wt_sb)):
                pj = big.tile([BN, F2], F32, name=f"proj_{nm}")
                pps = psum.tile([BN, F2], F32, space="PSUM", tag="ps")
                for kt in range(KA):
                    nc.tensor.matmul(pps[:],
                                     lhsT=entA_sb[:, kt * BN:(kt + 1) * BN],
                                     rhs=w_sb[:, kt * F2:(kt + 1) * F2],
                                     start=(kt == 0), stop=(kt == KA - 1))
                nc.any.tensor_copy(pj[:], pps[:])
                proj[nm] = pj

            # ---- hs pair-major = tanh(h_t + onehot_h.T @ P_head) ----
            hs = big.tile([128, PT * F2], F32)
            for t in range(PT):
                gps = psum.tile([128, F2], F32, space="PSUM", tag="ps")
                nc.tensor.matmul(gps[:],
                                 lhsT=ohh_sb[:, t * 128:(t + 1) * 128],
                                 rhs=proj["h"][:],
                                 start=True, stop=True)
                tmp = work.tile([128, F2], F32, tag="tmp_hs")
                nc.vector.tensor_tensor(out=tmp[:], in0=gps[:],
                                        in1=h_t[:, t * F2:(t + 1) * F2], op=ALU.add)
                nc.scalar.activation(hs[:, t * F2:(t + 1) * F2], tmp[:], ACTF.Tanh)

            # ---- ts transposed = tanh(h_tT + P_tail.T-gather), cast to bf16 ----
            tsT = [big.tile([128, P3_PAD], BF16, name=f"tsT{m}") for m in range(2)]
            for m in range(2):
                for nchk in range(PT // 4 + 1):
                    n0, n1 = nchk * 512, min((nchk + 1) * 512, P3_PAD)
                    if n0 >= n1:
                        continue
                    gps2 = psum.tile([128, 512], F32, space="PSUM", tag="ps")
                    nc.tensor.matmul(gps2[:, :n1 - n0],
                                     lhsT=proj["t"][:, m * 128:(m + 1) * 128],
                                     rhs=oht_sb[:, n0:n1],
                                     start=True, stop=True)
                    tmp2 = work.tile([128, 512], F32, tag="tmp_ts")
                    nc.vector.tensor_tensor(out=tmp2[:, :n1 - n0],
                                            in0=gps2[:, :n1 - n0],
                                            in1=h_tT[m][:, n0:n1], op=ALU.add)
                    nc.scalar.activation(tsT[m][:, n0:n1], tmp2[:, :n1 - n0],
                                         ACTF.Tanh)

            # ---- bilinear: stage-1 on PE, stage-2 fused on DVE ----
            lg_sb = big.tile([128, PT * NO], F32)
            NGRP = (NO + 1) // 2  # 7 groups of <=2 channels (one PSUM bank each)
            for t in range(PT):
                for grp in range(NGRP):
                    o0 = grp * 2
                    no = min(2, NO - o0)
                    rps = rpsum.tile([128, 512], F32, space="PSUM", tag="rps")
                    for j in range(2):
                        nc.tensor.matmul(
                            rps[:, :no * F2],
                            lhsT=tsT[j][:, t * 128:(t + 1) * 128],
                            rhs=wbil_sb[j][:, o0 * F2:(o0 + no) * F2],
                            start=(j == 0), stop=(j == 1))
                    for oo in range(no):
                        o = o0 + oo
                        scr = work.tile([128, F2], F32, tag="scr")
                        nc.vector.scalar_tensor_tensor(
                            out=scr[:], in0=rps[:, oo * F2:(oo + 1) * F2],
                            scalar=1.0, in1=hs[:, t * F2:(t + 1) * F2],
                            op0=ALU.mult, op1=ALU.mult,
                            accum_out=lg_sb[:, t * NO + o: t * NO + o + 1])
            # + b_bil (broadcast over pair tiles)
            lgv = lg_sb[:].rearrange("p (t o) -> p t o", o=NO)
            nc.vector.tensor_tensor(
                out=lgv, in0=lgv,
                in1=bbil_sb[:].unsqueeze(1).to_broadcast([128, PT, NO]),
                op=ALU.add)
            nc.sync.dma_start(
                out=lg_out[:].rearrange("(t p) o -> p t o", p=128),
                in_=lg_sb[:].rearrange("p (t o) -> p t o", o=NO))
    nc.compile()
    return nc


# ---------------------------------------------------------------------------
# Host orchestration
# ---------------------------------------------------------------------------

_CACHE = {}
LAST_EXEC_NS = []


def _get_programs():
    if "nc1" not in _CACHE:
        _CACHE["nc1"] = build_launch1()
        _CACHE["nc2"] = build_launch2()
    return _CACHE["nc1"], _CACHE["nc2"]


def _install_profile_hook():
    """The agent image's antenv lacks axon_hooks; synthesize it and register
    the ctypes NTFF hook from trn_agent_boot so trace=True can measure HW
    exec time. Also stub out the artifact upload (no bucket access here)."""
    if _CACHE.get("hook_done"):
        return
    import types
    import antenv

    mod = types.ModuleType("antenv.axon_hooks")
    mod._hook = None
    mod.set_axon_ntff_profile_hook = lambda h: setattr(mod, "_hook", h)
    mod.get_axon_ntff_profile_hook = lambda: mod._hook
    sys.modules["antenv.axon_hooks"] = mod
    antenv.axon_hooks = mod
    try:
        from trn_agent_boot.trn_boot import _ntff_profile_via_ctypes
        mod._hook = _ntff_profile_via_ctypes("/opt/axon/libaxon_pjrt.so")
    except Exception as e:  # pragma: no cover
        print(f"NTFF hook unavailable: {e}")
    bass_utils.upload_artifacts = lambda tmpdir: f"file://{tmpdir}"
    _CACHE["hook_done"] = True


def _run(nc, in_maps, tag):
    trace = bool(int(os.environ.get("KERNEL_TRACE", "0")))
    print(f"[kernel] running {tag} (trace={trace})", flush=True)
    if trace:
        _install_profile_hook()
    res = bass_utils.run_bass_kernel_spmd(nc, in_maps, list(range(NCORES)),
                                          trace=trace)
    print(f"[kernel] {tag} done exec_ns={res.exec_time_ns}", flush=True)
    if res.exec_time_ns is not None:
        LAST_EXEC_NS.append((tag, res.exec_time_ns, res.max_exec_time_core_id))
    return res.results


def prep1(sequence_output, attention, mention_idx, mention_mask, W_lin):
    ident = np.eye(128, dtype=np.float32)
    wlin4 = np.zeros((D, 4), np.float32)
    wlin4[:, :3] = W_lin
    maps1 = []
    for c in range(NCORES):
        b, q = c // 4, c % 4
        ls = q * LS
        att_sl = np.ascontiguousarray(
            attention[b, :, :, ls:ls + LS].transpose(1, 0, 2)
        ).reshape(L, H * LS).astype(np_bf16)
        seqT_sl = np.ascontiguousarray(sequence_output[b].T[:, ls:ls + LS])

        mi = mention_idx[b]      # [NE, M]
        mk = mention_mask[b]     # [NE, M]
        mi_pad = np.zeros((NEP, MM), np.int64)
        mi_pad[:NE] = mi
        mk_pad = np.zeros((NEP, MM), np.float32)
        mk_pad[:NE] = mk
        mk_pad[NE:, 0] = 1.0  # keep one live slot so pad logsumexp stays finite

        # shared row gather order: d = g*128 + (ne_sub*8+m)
        mg = mi_pad.reshape(-1)

        # mask-mean weights [128, NEP]
        wm = np.zeros((128, NEP), np.float32)
        cnt = np.maximum(mk_pad.sum(1), 1e-9)
        for ne in range(NEP):
            g, ne_sub = ne // 16, ne % 16
            wm[ne_sub * 8:(ne_sub + 1) * 8, ne] = mk_pad[ne] / cnt[ne]
        # NOTE: rows of wm are within-group (g) partitions; entity column ne only
        # draws from its own group's gather block because matmuls are done per g.

        am = np.broadcast_to(
            np.where(mk_pad.reshape(-1) > 0, 0.0, -1e30).astype(np.float32),
            (128, NEP * MM)).copy()

        maps1.append(dict(
            att=att_sl, seq=np.ascontiguousarray(sequence_output[b]),
            seqT=seqT_sl, wlin=wlin4,
            wmsk=wm.astype(np_bf16), amask=am,
            midx=_wrap_idx16(mg, NG * 128), ident=ident,
            identb=ident.astype(np_bf16)))
    return maps1


def prep2(res1, hts, W_lin, b_lin, W_seg, b_seg, W_head, b_head,
          W_tail, b_tail, W_bil, b_bil):
    ident = np.eye(128, dtype=np.float32)
    # ---- host resharding glue ----
    ai_full = np.zeros((B, NU_PAD, 4), np.float32)
    for c in range(NCORES):
        ai_full[c // 4] += res1[c]["ai_part"]
    entT = np.stack([res1[0]["ent_T"], res1[4]["ent_T"]])  # [B, D, NEP]

    # expand unique-pair table to hts order
    flat_u = _pair_u(hts[:, :, 0].reshape(-1), hts[:, :, 1].reshape(-1))
    bidx = np.repeat(np.arange(B), NP)
    ai_pairs = ai_full[bidx, flat_u]                       # [P3, 4]
    ai_pairs = np.concatenate(
        [ai_pairs, np.zeros((P3_PAD - P3, 4), np.float32)], 0)

    # augmented operands (bias folding)
    entA = np.zeros((DA, BN), np.float32)
    for b in range(B):
        entA[:D, b * NEP:(b + 1) * NEP] = entT[b]
    entA[D, :] = 1.0
    wheadA = np.zeros((DA, F2), np.float32)
    wheadA[:D] = W_head
    wheadA[D] = b_head
    wtailA = np.zeros((DA, F2), np.float32)
    wtailA[:D] = W_tail
    wtailA[D] = b_tail
    wsegA = np.concatenate([W_seg, (b_lin @ W_seg + b_seg)[None]], 0)  # [4, F2]

    # pair one-hots [BN, P3_PAD]
    ohh = np.zeros((BN, P3_PAD), np.float32)
    oht = np.zeros((BN, P3_PAD), np.float32)
    p_arange = np.arange(P3)
    ohh[bidx * NEP + hts[:, :, 0].reshape(-1), p_arange] = 1.0
    oht[bidx * NEP + hts[:, :, 1].reshape(-1), p_arange] = 1.0

    maps2 = []
    for c in range(NCORES):
        o0 = c * NO
        wb = np.zeros((F2, NO * F2), np.float32)   # [j, (o, i)]  (sent as bf16)
        bb = np.zeros((NO,), np.float32)
        no = max(0, min(NO, C - o0))
        if no > 0:
            # W_bil[o, i, j] -> [j, o, i]
            wb[:, :no * F2] = np.ascontiguousarray(
                W_bil[o0:o0 + no].transpose(2, 0, 1)).reshape(F2, no * F2)
            bb[:no] = b_bil[o0:o0 + no]
        maps2.append(dict(
            ai_pairs=ai_pairs, entA=entA, whead=wheadA, wtail=wtailA,
            wseg=wsegA, oh_h=ohh, oh_t=oht, wbil=wb.astype(np_bf16),
            bbil=np.broadcast_to(bb, (128, NO)).copy(), ident=ident))
    return maps2


def assemble(res2):
    logits = np.zeros((P3, C), np.float32)
    for c in range(NCORES):
        o0 = c * NO
        no = max(0, min(NO, C - o0))
        if no > 0:
            logits[:, o0:o0 + no] = res2[c]["logits_part"][:P3, :no]
    return logits


def kernel(sequence_output, attention, mention_idx, mention_mask, hts,
           W_lin, b_lin, W_seg, b_seg, W_head, b_head, W_tail, b_tail,
           W_bil, b_bil):
    sequence_output = np.asarray(sequence_output, np.float32)
    attention = np.asarray(attention, np.float32)
    mention_idx = np.asarray(mention_idx, np.int32)
    mention_mask = np.asarray(mention_mask, np.int32)
    hts = np.asarray(hts, np.int32)
    args = [np.asarray(a, np.float32) for a in
            (W_lin, b_lin, W_seg, b_seg, W_head, b_head, W_tail, b_tail,
             W_bil, b_bil)]
    (W_lin, b_lin, W_seg, b_seg, W_head, b_head, W_tail, b_tail,
     W_bil, b_bil) = args

    LAST_EXEC_NS.clear()
    nc1, nc2 = _get_programs()
    maps1 = prep1(sequence_output, attention, mention_idx, mention_mask, W_lin)
    res1 = _run(nc1, maps1, "launch1")
    maps2 = prep2(res1, hts, W_lin, b_lin, W_seg, b_seg, W_head, b_head,
                  W_tail, b_tail, W_bil, b_bil)
    res2 = _run(nc2, maps2, "launch2")
    return assemble(res2)



# revision 9
# speedup vs baseline: 2.4102x; 2.4102x over previous
"""Trainium2 Bass kernel for nn_DocREModel (DocRE-style relation extraction head).

Strategy (8 NeuronCores, two SPMD launches):

Launch 1  (core c -> batch b=c//4, l-slice q=c%4 of 256 positions):
  - dma_gather mention rows of attention[b,:,:,lslice]; masked-mean via a
    3-ktile matmul -> E [48, H*LS] bf16; PE-transpose -> E_T[lt] [l, (h,i)].
  - seqW = seq[lslice] @ [W_lin|1]/H.
  - Weighted Gram on PE: T_x[i,j] = sum_{h,l} E_T[l,hi] * (E_T*w_x)[l,hj]
    -> all 48x48 pair features in 96 small matmuls (T is symmetric; both
    orderings come for free).  Output T_out [48, 4*48] per core (partial
    over l; host sums the 4 l-slices).
  - mention gather of a D-slice (192 cols, q-th quarter) of sequence_output
    + masked logsumexp -> entd [192, 48] (D-split across the batch's cores).

Launch 2  (core c -> channel slice of 13 of the 97 bilinear channels,
           pairs deduped to the unique (b,h,t) triples, padded to PT2*128):
  - host sends aiT [4, PQ] = normalized [T_0,T_1,T_2,1] per unique pair.
  - h_t = relu(wseg^T aiT) pair-major and transposed (bf16 matmuls).
  - hs = tanh(onehot_h @ P_head + h_t) via a 2-ktile matmul trick
    (identity ktile adds the pair-major h_t); same for ts^T, output bf16.
  - bilinear stage 1 on PE (bf16, N=512 banks), stage 2 split across
    DVE (scalar_tensor_tensor from PSUM) and an ACT-copy + DVE-multiply +
    ACT-accumulate path for the remaining channels.
  - host scatters unique-pair logits back to the 3444 (b,p) rows.
"""

import os
import sys

for _p in ("/opt/trn_rl_repo", "/root/.axon_site/_ro/trn_rl_repo"):
    if os.path.isdir(_p) and _p not in sys.path:
        sys.path.append(_p)

import numpy as np
from ml_dtypes import bfloat16 as np_bf16

from concourse import bacc, bass, mybir, tile
from concourse import bass_utils

F32 = mybir.dt.float32
F32R = mybir.dt.float32r
BF16 = mybir.dt.bfloat16
I16 = mybir.dt.int16
ALU = mybir.AluOpType
ACTF = mybir.ActivationFunctionType
AXX = mybir.AxisListType.X

# Problem shape (hardcoded per the harness contract).
B, L, D, H, NE, MM, NP, C, F2 = 2, 1024, 768, 12, 42, 8, 1722, 97, 256
NCORES = 8
LS = L // 4            # 256 l-positions per launch-1 core
DS = D // 4            # 192 D-slice per launch-1 core (for ent logsumexp)
NEP = 48               # padded entity count
NG = 3                 # gather groups of 128 mention rows
NMEN = NG * 128        # 384 gathered rows
KD = D // 128          # 6
HLS = H * LS           # 3072
HNE = H * NEP          # 576
NO = 13                # channels per launch-2 core (8*13 = 104 >= 97)
DA = 896               # 768 + bias row, padded to 7*128
BN = 2 * NEP           # 96 (batch, entity) columns
KA = DA // 128         # 7


def _wrap_idx16(idx, n):
    """Pack indices into the [128, n//16] int16 layout dma_gather expects."""
    assert len(idx) == n and n % 16 == 0
    out = np.zeros((16, n // 16), dtype=np.int16)
    out[np.arange(n) % 16, np.arange(n) // 16] = idx
    return np.tile(out, (8, 1))


# ---------------------------------------------------------------------------
# Launch 1 program
# ---------------------------------------------------------------------------

def build_launch1():
    nc = bacc.Bacc("TRN2", target_bir_lowering=False, debug=False)
    att = nc.declare_dram_parameter("att", [L, HLS], BF16, isOutput=False)
    seqd = nc.declare_dram_parameter("seqd", [L, DS], F32, isOutput=False)
    seqT = nc.declare_dram_parameter("seqT", [D, LS], F32, isOutput=False)
    wlin = nc.declare_dram_parameter("wlin", [D, 4], F32, isOutput=False)
    wmsk3 = nc.declare_dram_parameter("wmsk3", [NMEN, NEP], BF16, isOutput=False)
    amask = nc.declare_dram_parameter("amask", [128, NMEN], F32, isOutput=False)
    midx = nc.declare_dram_parameter("midx", [128, NMEN // 16], I16, isOutput=False)
    identb = nc.declare_dram_parameter("identb", [128, 128], BF16, isOutput=False)
    identf = nc.declare_dram_parameter("identf", [128, 128], F32, isOutput=False)
    T_out = nc.declare_dram_parameter("T_out", [NEP, 4 * NEP], F32, isOutput=True)
    ent_out = nc.declare_dram_parameter("entd", [DS, NEP], F32, isOutput=True)

    with tile.TileContext(nc) as tc:
        with (
            tc.tile_pool(name="big", bufs=1) as big,
            tc.tile_pool(name="small", bufs=1) as small,
            tc.tile_pool(name="work", bufs=2) as work,
            tc.tile_pool(name="psA", bufs=2, space="PSUM") as psA,
            tc.tile_pool(name="psB", bufs=2, space="PSUM") as psB,
            tc.tile_pool(name="psT", bufs=1, space="PSUM") as psT,
        ):
            att_rows = big.tile([128, NG * HLS], BF16)
            ment_rows = big.tile([128, NG * DS], F32)
            seqT_sb = big.tile([128, KD * LS], F32)
            wlin_sb = small.tile([128, KD * 4], F32)
            wmsk_sb = small.tile([128, NG * NEP], BF16)
            amask_sb = small.tile([128, NMEN], F32)
            midx_sb = small.tile([128, NMEN // 16], I16)
            identb_sb = small.tile([128, 128], BF16)
            identf_sb = small.tile([128, 128], F32)

            # midx first so the ragged gathers issue as early as possible
            nc.sync.dma_start(out=midx_sb[:], in_=midx[:])
            nc.gpsimd.dma_gather(
                out_ap=att_rows[:].rearrange("p (c l) -> p c l", l=HLS),
                in_ap=att[:], idxs_ap=midx_sb[:],
                num_idxs=NMEN, num_idxs_reg=NMEN, elem_size=HLS,
                single_packet=False)
            nc.gpsimd.dma_gather(
                out_ap=ment_rows[:].rearrange("p (c l) -> p c l", l=DS),
                in_ap=seqd[:], idxs_ap=midx_sb[:],
                num_idxs=NMEN, num_idxs_reg=NMEN, elem_size=DS,
                single_packet=False)
            nc.sync.dma_start(out=wmsk_sb[:].rearrange("p (g e) -> p g e", g=NG),
                              in_=wmsk3[:].rearrange("(g p) e -> p g e", p=128))
            nc.sync.dma_start(out=identb_sb[:], in_=identb[:])
            nc.sync.dma_start(out=identf_sb[:], in_=identf[:])
            nc.sync.dma_start(out=seqT_sb[:].rearrange("p (k l) -> p k l", k=KD),
                              in_=seqT[:].rearrange("(k p) l -> p k l", p=128))
            nc.sync.dma_start(out=wlin_sb[:].rearrange("p (k x) -> p k x", k=KD),
                              in_=wlin[:].rearrange("(k p) x -> p k x", p=128))
            nc.sync.dma_start(out=amask_sb[:], in_=amask[:])

            # ---- masked mean over mentions: E [48, H*LS] ----
            E_sb = big.tile([NEP, HLS], BF16)
            for ch in range(HLS // 512):
                eps = psA.tile([NEP, 512], F32, space="PSUM", tag="ps1")
                for g in range(NG):
                    nc.tensor.matmul(
                        eps[:], lhsT=wmsk_sb[:, g * NEP:(g + 1) * NEP],
                        rhs=att_rows[:, g * HLS + ch * 512: g * HLS + (ch + 1) * 512],
                        start=(g == 0), stop=(g == NG - 1))
                nc.scalar.activation(E_sb[:, ch * 512:(ch + 1) * 512], eps[:],
                                     ACTF.Copy)

            # ---- transpose E -> E_T[lt][l, (h,i)] ----
            E_T = [big.tile([128, HNE], BF16, name=f"E_T{lt}") for lt in range(2)]
            for h in range(H):
                for lt in range(2):
                    tps = psB.tile([128, NEP], BF16, space="PSUM", tag="ps2")
                    nc.tensor.transpose(
                        tps[:], E_sb[:, h * LS + lt * 128: h * LS + (lt + 1) * 128],
                        identb_sb[:NEP, :NEP])
                    nc.vector.tensor_copy(E_T[lt][:, h * NEP:(h + 1) * NEP], tps[:])

            # ---- seqW = seqT.T @ [W_lin|e]/H ----
            seqW = [small.tile([128, 4], F32, name=f"seqW{lt}") for lt in range(2)]
            for lt in range(2):
                swps = psA.tile([128, 4], F32, space="PSUM", tag="ps1")
                for kt in range(KD):
                    nc.tensor.matmul(
                        swps[:],
                        lhsT=seqT_sb[:, kt * LS + lt * 128: kt * LS + (lt + 1) * 128],
                        rhs=wlin_sb[:, kt * 4:(kt + 1) * 4],
                        start=(kt == 0), stop=(kt == KD - 1))
                nc.scalar.activation(seqW[lt][:], swps[:], ACTF.Copy, scale=1.0 / H)
                nc.vector.memset(seqW[lt][:, 3:4], 1.0 / H)

            # ---- rhs_x = E_T * w_x (per-partition scale on ACT engine) ----
            rhsx = [big.tile([128, 4 * HNE], BF16, name=f"rhsx{lt}")
                    for lt in range(2)]
            for lt in range(2):
                for x in range(4):
                    nc.vector.tensor_scalar(
                        out=rhsx[lt][:, x * HNE:(x + 1) * HNE], in0=E_T[lt][:],
                        scalar1=seqW[lt][:, x:x + 1], scalar2=None,
                        op0=ALU.mult)

            # ---- weighted Gram: T_x = sum_h E_h diag(w_x) E_h^T ----
            T4 = psT.tile([NEP, 4 * NEP], F32, space="PSUM")
            for x in range(4):
                for lt in range(2):
                    for h in range(H):
                        nc.tensor.matmul(
                            T4[:, x * NEP:(x + 1) * NEP],
                            lhsT=E_T[lt][:, h * NEP:(h + 1) * NEP],
                            rhs=rhsx[lt][:, x * HNE + h * NEP: x * HNE + (h + 1) * NEP],
                            start=(lt == 0 and h == 0),
                            stop=(lt == 1 and h == H - 1))
            T_sb = small.tile([NEP, 4 * NEP], F32)
            nc.vector.tensor_copy(T_sb[:], T4[:])
            nc.sync.dma_start(out=T_out[:], in_=T_sb[:])

            # ---- mention transposes + masked logsumexp over the D-slice ----
            ent_sb = big.tile([96, 2 * NEP], F32)
            for half in range(2):
                mT = work.tile([96, NMEN], F32, tag="mT")
                for g in range(NG):
                    mps = psB.tile([96, 128], F32, space="PSUM", tag="ps2")
                    nc.tensor.transpose(
                        mps[:],
                        ment_rows[:, g * DS + half * 96: g * DS + half * 96 + 96],
                        identf_sb[:])
                    nc.vector.tensor_copy(mT[:, g * 128:(g + 1) * 128], mps[:])
                xm = work.tile([96, NMEN], F32, tag="xm")
                nc.vector.tensor_tensor(out=xm[:], in0=mT[:], in1=amask_sb[:96, :],
                                        op=ALU.add)
                xmv = xm[:].rearrange("p (e m) -> p e m", m=MM)
                mx = work.tile([96, NEP], F32, tag="mx")
                nc.vector.tensor_reduce(out=mx[:], in_=xmv, axis=AXX, op=ALU.max)
                xs = work.tile([96, NMEN], F32, tag="xs")
                nc.vector.tensor_tensor(
                    out=xs[:].rearrange("p (e m) -> p e m", m=MM), in0=xmv,
                    in1=mx[:].unsqueeze(2).to_broadcast([96, NEP, MM]),
                    op=ALU.subtract)
                es = work.tile([96, NMEN], F32, tag="es")
                nc.scalar.activation(es[:], xs[:], ACTF.Exp)
                sm = work.tile([96, NEP], F32, tag="sm")
                nc.vector.tensor_reduce(
                    out=sm[:], in_=es[:].rearrange("p (e m) -> p e m", m=MM),
                    axis=AXX, op=ALU.add)
                ln = work.tile([96, NEP], F32, tag="ln")
                nc.scalar.activation(ln[:], sm[:], ACTF.Ln)
                nc.vector.tensor_tensor(
                    out=ent_sb[:, half * NEP:(half + 1) * NEP], in0=ln[:],
                    in1=mx[:], op=ALU.add)
            nc.sync.dma_start(
                out=ent_out[:].rearrange("(h p) e -> p h e", p=96),
                in_=ent_sb[:].rearrange("p (h e) -> p h e", e=NEP))
    return nc


# ---------------------------------------------------------------------------
# Launch 2 program (parameterized by the unique-pair tile count PT2)
# ---------------------------------------------------------------------------

def build_launch2(PT2):
    PQ = PT2 * 128
    nc = bacc.Bacc("TRN2", target_bir_lowering=False, debug=False)
    aiT = nc.declare_dram_parameter("aiT", [4, PQ], BF16, isOutput=False)
    entA = nc.declare_dram_parameter("entA", [DA, BN], BF16, isOutput=False)
    whead = nc.declare_dram_parameter("whead", [DA, F2], BF16, isOutput=False)
    wtail = nc.declare_dram_parameter("wtail", [DA, F2], BF16, isOutput=False)
    wseg = nc.declare_dram_parameter("wseg", [4, F2], BF16, isOutput=False)
    oh_h = nc.declare_dram_parameter("oh_h", [BN, PQ], BF16, isOutput=False)
    oh_t = nc.declare_dram_parameter("oh_t", [BN, PQ], BF16, isOutput=False)
    wbil = nc.declare_dram_parameter("wbil", [F2, NO * F2], BF16, isOutput=False)
    bbil = nc.declare_dram_parameter("bbil", [128, NO], F32, isOutput=False)
    identf = nc.declare_dram_parameter("identf", [128, 128], BF16, isOutput=False)
    lg_out = nc.declare_dram_parameter("logits_part", [PQ, NO], F32, isOutput=True)

    NBK = (NO + 1) // 2  # 7 PSUM banks per pair tile
    NCH = (PQ + 511) // 512  # 512-chunks over pairs

    with tile.TileContext(nc) as tc:
        with (
            tc.tile_pool(name="big", bufs=1) as big,
            tc.tile_pool(name="small", bufs=1) as small,
            tc.tile_pool(name="work", bufs=3) as work,
            tc.tile_pool(name="psP", bufs=2, space="PSUM") as psP,
            tc.tile_pool(name="rb", bufs=5, space="PSUM") as rb,
        ):
            aiT_sb = big.tile([4, PQ], BF16)
            entA_sb = big.tile([128, KA * BN], BF16)
            wh_sb = big.tile([128, KA * F2], BF16)
            wt_sb = big.tile([128, KA * F2], BF16)
            wseg_sb = small.tile([4, F2], BF16)
            ohh_sb = big.tile([BN, PQ], BF16)
            oht_sb = big.tile([BN, PQ], BF16)
            wbil_sb = big.tile([128, 2 * NO * F2], BF16)
            bbil_sb = small.tile([128, NO], F32)
            id_sb = small.tile([128, 128], BF16)

            nc.sync.dma_start(out=aiT_sb[:], in_=aiT[:])
            nc.sync.dma_start(out=wseg_sb[:], in_=wseg[:])
            nc.sync.dma_start(out=entA_sb[:].rearrange("p (k n) -> p k n", k=KA),
                              in_=entA[:].rearrange("(k p) n -> p k n", p=128))
            nc.sync.dma_start(out=wh_sb[:].rearrange("p (k f) -> p k f", k=KA),
                              in_=whead[:].rearrange("(k p) f -> p k f", p=128))
            nc.sync.dma_start(out=wt_sb[:].rearrange("p (k f) -> p k f", k=KA),
                              in_=wtail[:].rearrange("(k p) f -> p k f", p=128))
            nc.sync.dma_start(out=ohh_sb[:], in_=oh_h[:])
            nc.sync.dma_start(out=oht_sb[:], in_=oh_t[:])
            nc.sync.dma_start(out=id_sb[:], in_=identf[:])
            nc.sync.dma_start(out=bbil_sb[:], in_=bbil[:])
            nc.sync.dma_start(out=wbil_sb[:].rearrange("p (j f) -> p j f", j=2),
                              in_=wbil[:].rearrange("(j p) f -> p j f", p=128))

            # ---- h_t pair-major [p, F2] = relu(wseg^T @ aiT) ----
            h_t = big.tile([128, PT2 * F2], BF16)
            for t in range(PT2):
                hps = psP.tile([128, F2], F32, space="PSUM", tag="ps")
                nc.tensor.matmul(hps[:], lhsT=aiT_sb[:, t * 128:(t + 1) * 128],
                                 rhs=wseg_sb[:], start=True, stop=True)
                nc.scalar.activation(h_t[:, t * F2:(t + 1) * F2], hps[:], ACTF.Relu)

            # ---- h_tT [f, p] halves ----
            h_tT = [big.tile([128, PQ], BF16, name=f"h_tT{m}") for m in range(2)]
            for m in range(2):
                for ch in range(NCH):
                    n0, n1 = ch * 512, min((ch + 1) * 512, PQ)
                    hps2 = psP.tile([128, 512], F32, space="PSUM", tag="ps")
                    nc.tensor.matmul(hps2[:, :n1 - n0],
                                     lhsT=wseg_sb[:, m * 128:(m + 1) * 128],
                                     rhs=aiT_sb[:, n0:n1],
                                     start=True, stop=True)
                    nc.scalar.activation(h_tT[m][:, n0:n1], hps2[:, :n1 - n0],
                                         ACTF.Relu)

            # ---- projections P_head/P_tail [bn, F2] ----
            proj = {}
            for nm, w_sb in (("h", wh_sb), ("t", wt_sb)):
                pj = big.tile([BN, F2], BF16, name=f"proj_{nm}")
                pps = psP.tile([BN, F2], F32, space="PSUM", tag="ps")
                for kt in range(KA):
                    nc.tensor.matmul(pps[:],
                                     lhsT=entA_sb[:, kt * BN:(kt + 1) * BN],
                                     rhs=w_sb[:, kt * F2:(kt + 1) * F2],
                                     start=(kt == 0), stop=(kt == KA - 1))
                nc.vector.tensor_copy(pj[:], pps[:])
                proj[nm] = pj

            # ---- hs pair-major bf16 = tanh(onehot_h @ P_head + h_t) ----
            hs = big.tile([128, PT2 * F2], BF16)
            for t in range(PT2):
                gps = psP.tile([128, F2], F32, space="PSUM", tag="ps")
                nc.tensor.matmul(gps[:], lhsT=ohh_sb[:, t * 128:(t + 1) * 128],
                                 rhs=proj["h"][:], start=True, stop=False)
                nc.tensor.matmul(gps[:], lhsT=id_sb[:],
                                 rhs=h_t[:, t * F2:(t + 1) * F2],
                                 start=False, stop=True)
                nc.scalar.activation(hs[:, t * F2:(t + 1) * F2], gps[:], ACTF.Tanh)

            # ---- tsT bf16 = tanh(P_tail^T gather + h_tT) ----
            tsT = [big.tile([128, PQ], BF16, name=f"tsT{m}") for m in range(2)]
            for m in range(2):
                for ch in range(NCH):
                    n0, n1 = ch * 512, min((ch + 1) * 512, PQ)
                    gps2 = psP.tile([128, 512], F32, space="PSUM", tag="ps")
                    nc.tensor.matmul(gps2[:, :n1 - n0],
                                     lhsT=proj["t"][:, m * 128:(m + 1) * 128],
                                     rhs=oht_sb[:, n0:n1],
                                     start=True, stop=False)
                    nc.tensor.matmul(gps2[:, :n1 - n0], lhsT=id_sb[:],
                                     rhs=h_tT[m][:, n0:n1],
                                     start=False, stop=True)
                    nc.scalar.activation(tsT[m][:, n0:n1], gps2[:, :n1 - n0],
                                         ACTF.Tanh)

            # ---- bilinear: stage 1 on PE, stage 2 split DVE / ACT+GpSimd ----
            lg_sb = big.tile([128, PT2 * NO], F32)
            DVE_BANKS = (0, 1, 2, 3, 6)
            GS_BANKS = (4, 5)
            for t in range(PT2):
                banks = []
                for g in range(NBK):
                    nw = min(2, NO - 2 * g) * F2
                    banks.append(rb.tile([128, nw], F32, space="PSUM",
                                         tag="rb", name=f"rb_{t}_{g}"))
                for gchunk in ((0, 1, 2, 3), (4, 5, 6)):
                    for j in range(2):
                        for g in gchunk:
                            nw = min(2, NO - 2 * g) * F2
                            nc.tensor.matmul(
                                banks[g][:],
                                lhsT=tsT[j][:, t * 128:(t + 1) * 128],
                                rhs=wbil_sb[:, j * NO * F2 + 2 * g * F2:
                                            j * NO * F2 + 2 * g * F2 + nw],
                                start=(j == 0), stop=(j == 1))
                for g in DVE_BANKS:
                    for oo in range(min(2, NO - 2 * g)):
                        o = 2 * g + oo
                        scr = work.tile([128, F2], F32, tag="scr")
                        nc.vector.scalar_tensor_tensor(
                            out=scr[:], in0=banks[g][:, oo * F2:(oo + 1) * F2],
                            scalar=1.0, in1=hs[:, t * F2:(t + 1) * F2],
                            op0=ALU.mult, op1=ALU.mult,
                            accum_out=lg_sb[:, t * NO + o: t * NO + o + 1])
                for g in GS_BANKS:
                    Rc = work.tile([128, 2 * F2], BF16, tag="rc")
                    nc.scalar.activation(Rc[:], banks[g][:], ACTF.Copy)
                    prod = work.tile([128, 2 * F2], BF16, tag="prod")
                    nc.vector.tensor_tensor(
                        out=prod[:].rearrange("p (c f) -> p c f", c=2),
                        in0=Rc[:].rearrange("p (c f) -> p c f", c=2),
                        in1=hs[:, t * F2:(t + 1) * F2].unsqueeze(1)
                            .to_broadcast([128, 2, F2]),
                        op=ALU.mult)
                    for oo in range(2):
                        o = 2 * g + oo
                        scr2 = work.tile([128, F2], BF16, tag="sc2")
                        nc.scalar.activation(
                            scr2[:], prod[:, oo * F2:(oo + 1) * F2], ACTF.Copy,
                            accum_out=lg_sb[:, t * NO + o: t * NO + o + 1])

            # + b_bil (broadcast over pair tiles)
            lgv = lg_sb[:].rearrange("p (t o) -> p t o", o=NO)
            nc.vector.tensor_tensor(
                out=lgv, in0=lgv,
                in1=bbil_sb[:].unsqueeze(1).to_broadcast([128, PT2, NO]),
                op=ALU.add)
            nc.sync.dma_start(
                out=lg_out[:].rearrange("(t p) o -> p t o", p=128),
                in_=lg_sb[:].rearrange("p (t o) -> p t o", o=NO))
    return nc


# ---------------------------------------------------------------------------
# Host orchestration
# ---------------------------------------------------------------------------

_CACHE = {}
LAST_EXEC_NS = []


def _get_launch1():
    if "nc1" not in _CACHE:
        nc = build_launch1()
        _CACHE["nc1"] = nc
    return _CACHE["nc1"]


def _get_launch2(PT2):
    key = f"nc2_{PT2}"
    if key not in _CACHE:
        _CACHE[key] = build_launch2(PT2)
    return _CACHE[key]


def _install_profile_hook():
    """Synthesize antenv.axon_hooks + register the ctypes NTFF hook so
    trace=True can measure HW exec time (agent image lacks axon_hooks)."""
    if _CACHE.get("hook_done"):
        return
    import types
    import antenv

    mod = types.ModuleType("antenv.axon_hooks")
    mod._hook = None
    mod.set_axon_ntff_profile_hook = lambda h: setattr(mod, "_hook", h)
    mod.get_axon_ntff_profile_hook = lambda: mod._hook
    sys.modules["antenv.axon_hooks"] = mod
    antenv.axon_hooks = mod
    try:
        from trn_agent_boot.trn_boot import _ntff_profile_via_ctypes
        mod._hook = _ntff_profile_via_ctypes("/opt/axon/libaxon_pjrt.so")
    except Exception as e:  # pragma: no cover
        print(f"NTFF hook unavailable: {e}")
    bass_utils.upload_artifacts = lambda tmpdir: f"file://{tmpdir}"
    _CACHE["hook_done"] = True


_OUT_NAMES = {"launch1": ["T_out", "entd"], "launch2": ["logits_part"]}


def _run(nc, in_maps, tag):
    backend = os.environ.get("KERNEL_BACKEND", "hw")
    if backend == "sim":
        from concourse import bass_interp
        if not getattr(nc, "_lib_loads_done", False):
            nc.insert_library_loads()
            nc._lib_loads_done = True
        outs = []
        for m in in_maps:
            sim = bass_interp.CoreSim(nc)
            for k, v in m.items():
                sim.tensor(k)[:] = v
            sim.simulate()
            outs.append({o: np.array(sim.tensor(o)) for o in _OUT_NAMES[tag]})
        return outs
    trace = bool(int(os.environ.get("KERNEL_TRACE", "0")))
    print(f"[kernel] running {tag} (trace={trace})", flush=True)
    if trace:
        _install_profile_hook()
    if not getattr(nc, "_compiled_done", False):
        nc.compile()
        nc._compiled_done = True
    res = bass_utils.run_bass_kernel_spmd(nc, in_maps, list(range(NCORES)),
                                          trace=trace)
    print(f"[kernel] {tag} done exec_ns={res.exec_time_ns}", flush=True)
    if res.exec_time_ns is not None:
        LAST_EXEC_NS.append((tag, res.exec_time_ns, res.max_exec_time_core_id))
    return res.results


def prep1(sequence_output, attention, mention_idx, mention_mask, W_lin):
    identb = np.eye(128, dtype=np_bf16)
    identf = np.eye(128, dtype=np.float32)
    wlin4 = np.zeros((D, 4), np.float32)
    wlin4[:, :3] = W_lin
    maps1 = []
    for c in range(NCORES):
        b, q = c // 4, c % 4
        ls, ds = q * LS, q * DS
        att_sl = np.ascontiguousarray(
            attention[b, :, :, ls:ls + LS].transpose(1, 0, 2)
        ).reshape(L, HLS).astype(np_bf16)
        seqd_sl = np.ascontiguousarray(sequence_output[b][:, ds:ds + DS])
        seqT_sl = np.ascontiguousarray(sequence_output[b].T[:, ls:ls + LS])

        mi_pad = np.zeros((NEP, MM), np.int64)
        mi_pad[:NE] = mention_idx[b]
        mk_pad = np.zeros((NEP, MM), np.float32)
        mk_pad[:NE] = mention_mask[b]
        mk_pad[NE:, 0] = 1.0  # keep pad logsumexp finite

        mg = mi_pad.reshape(-1)  # row rg = ne*8 + m

        cnt = np.maximum(mk_pad.sum(1), 1e-9)
        wmsk3 = np.zeros((NMEN, NEP), np.float32)
        rows = np.arange(NMEN)
        wmsk3[rows, rows // MM] = (mk_pad / cnt[:, None]).reshape(-1)

        am = np.broadcast_to(
            np.where(mk_pad.reshape(-1) > 0, 0.0, -1e30).astype(np.float32),
            (128, NMEN)).copy()

        maps1.append(dict(
            att=att_sl, seqd=seqd_sl, seqT=seqT_sl, wlin=wlin4,
            wmsk3=wmsk3.astype(np_bf16), amask=am,
            midx=_wrap_idx16(mg, NMEN), identb=identb, identf=identf))
    return maps1


def _unique_pairs(hts):
    """Per-batch unique (h, t) pairs; returns (b_k, i_k, j_k, inv[B*NP])."""
    bs, is_, js = [], [], []
    inv = np.zeros(B * NP, np.int64)
    off = 0
    for b in range(B):
        keys = hts[b, :, 0].astype(np.int64) * NEP + hts[b, :, 1]
        u, invb = np.unique(keys, return_inverse=True)
        bs.append(np.full(len(u), b, np.int64))
        is_.append(u // NEP)
        js.append(u % NEP)
        inv[b * NP:(b + 1) * NP] = off + invb
        off += len(u)
    return (np.concatenate(bs), np.concatenate(is_), np.concatenate(js), inv)


def prep2(res1, hts, W_lin, b_lin, W_seg, b_seg, W_head, b_head,
          W_tail, b_tail, W_bil, b_bil):
    identf = np.eye(128, dtype=np.float32)
    # ---- host resharding glue ----
    T_full = np.zeros((B, NEP, 4, NEP), np.float32)
    ent_full = np.zeros((B, D, NEP), np.float32)
    for c in range(NCORES):
        b, q = c // 4, c % 4
        T_full[b] += res1[c]["T_out"].reshape(NEP, 4, NEP)
        ent_full[b, q * DS:(q + 1) * DS] = res1[c]["entd"]

    b_k, i_k, j_k, inv = _unique_pairs(hts)
    K = len(b_k)
    PT2 = -(-K // 128)
    PQ = PT2 * 128
    _CACHE["PT2"], _CACHE["inv"] = PT2, inv

    vals = T_full[b_k, i_k, :, j_k]                    # [K, 4]
    s = vals[:, 3] + 1e-5
    aiT = np.zeros((4, PQ), np.float32)
    aiT[:3, :K] = (vals[:, :3] / s[:, None]).T
    aiT[3, :K] = 1.0

    ohh = np.zeros((BN, PQ), np.float32)
    oht = np.zeros((BN, PQ), np.float32)
    ar = np.arange(K)
    ohh[b_k * NEP + i_k, ar] = 1.0
    oht[b_k * NEP + j_k, ar] = 1.0

    entA = np.zeros((DA, BN), np.float32)
    for b in range(B):
        entA[:D, b * NEP:(b + 1) * NEP] = ent_full[b]
    entA[D, :] = 1.0
    wheadA = np.zeros((DA, F2), np.float32)
    wheadA[:D] = W_head
    wheadA[D] = b_head
    wtailA = np.zeros((DA, F2), np.float32)
    wtailA[:D] = W_tail
    wtailA[D] = b_tail
    wsegA = np.concatenate([W_seg, (b_lin @ W_seg + b_seg)[None]], 0)  # [4, F2]

    maps2 = []
    for c in range(NCORES):
        o0 = c * NO
        wb = np.zeros((F2, NO * F2), np.float32)   # [j, (o, i)]
        bb = np.zeros((NO,), np.float32)
        no = max(0, min(NO, C - o0))
        if no > 0:
            wb[:, :no * F2] = np.ascontiguousarray(
                W_bil[o0:o0 + no].transpose(2, 0, 1)).reshape(F2, no * F2)
            bb[:no] = b_bil[o0:o0 + no]
        maps2.append(dict(
            aiT=aiT.astype(np_bf16), entA=entA.astype(np_bf16),
            whead=wheadA.astype(np_bf16), wtail=wtailA.astype(np_bf16),
            wseg=wsegA.astype(np_bf16),
            oh_h=ohh.astype(np_bf16), oh_t=oht.astype(np_bf16),
            wbil=wb.astype(np_bf16),
            bbil=np.broadcast_to(bb, (128, NO)).copy(),
            identf=np.eye(128, dtype=np_bf16)))
    return maps2


def assemble(res2):
    PT2, inv = _CACHE["PT2"], _CACHE["inv"]
    PQ = PT2 * 128
    logits_u = np.zeros((PQ, C), np.float32)
    for c in range(NCORES):
        o0 = c * NO
        no = max(0, min(NO, C - o0))
        if no > 0:
            logits_u[:, o0:o0 + no] = res2[c]["logits_part"][:, :no]
    return np.ascontiguousarray(logits_u[inv])


def kernel(sequence_output, attention, mention_idx, mention_mask, hts,
           W_lin, b_lin, W_seg, b_seg, W_head, b_head, W_tail, b_tail,
           W_bil, b_bil):
    sequence_output = np.asarray(sequence_output, np.float32)
    attention = np.asarray(attention, np.float32)
    mention_idx = np.asarray(mention_idx, np.int32)
    mention_mask = np.asarray(mention_mask, np.int32)
    hts = np.asarray(hts, np.int32)
    args = [np.asarray(a, np.float32) for a in
            (W_lin, b_lin, W_seg, b_seg, W_head, b_head, W_tail, b_tail,
             W_bil, b_bil)]
    (W_lin, b_lin, W_seg, b_seg, W_head, b_head, W_tail, b_tail,
     W_bil, b_bil) = args

    LAST_EXEC_NS.clear()
    nc1 = _get_launch1()
    maps1 = prep1(sequence_output, attention, mention_idx, mention_mask, W_lin)
    res1 = _run(nc1, maps1, "launch1")
    maps2 = prep2(res1, hts, W_lin, b_lin, W_seg, b_seg, W_head, b_head,
                  W_tail, b_tail, W_bil, b_bil)
    nc2 = _get_launch2(_CACHE["PT2"])
    res2 = _run(nc2, maps2, "launch2")
    return assemble(res2)
